# revision 26
# baseline (speedup 1.0000x reference)
"""Channel-attention kernel for Trainium2 (8 NeuronCores, SPMD data-parallel).

out[b] = beta * softmax(rowmax(S) - S, axis=-1) @ x[b] + x[b],  S = x[b] @ x[b].T

Sharding: batch dim B=16 split as 2 batches per core across 8 cores.

The device computes feat = beta * softmax(...) @ x and returns it in bf16;
the final `+ x` runs on host in fp32 (exact), so when beta == 0 the kernel
returns x bit-exactly.  Device inputs are a host-pretransposed fp8 copy xT
(the S matmul contracts over the feature dim n, which must live on SBUF
partitions for the PE) and a natural-layout fp8 copy of x (the F matmul's
moving operand).

Both matmul phases run fp8 with perf_mode=DoubleRow: operands are viewed
as [128, 2, F] APs pairing adjacent k-tiles, so one instruction contracts
K=256 (half the matmul count of bf16).

Math note: softmax(max_row(S) - S) row-wise equals exp(minrow - S) / Z with
Z = sum_d exp(minrow - S).  beta/Z is folded into A before the second
matmul.
"""

from contextlib import ExitStack

import numpy as np
import ml_dtypes

N_CORES = 8
B, C, N = 16, 512, 4096
BPC = B // N_CORES  # batches per core
P = 128
MT = C // P  # 4 row-blocks of channels
KT = N // P  # 32 partition-tiles of xT
XT_CH = 8  # xT dma/dep chunks (4 k-tiles each)
NQ = N // 512  # 8 n-chunks for the second matmul
KD = C // P  # 4 d-chunks for the second matmul

_CACHE = {}


def _build_bass(reps=1, loop_iters=1, dma_only=False, timing=False, mm2x=False,
                no_store=False):
    import concourse.bass as bass
    import concourse.bacc as bacc
    import concourse.mybir as mybir
    from concourse import tile, masks

    dt = mybir.dt
    AF = mybir.ActivationFunctionType
    ALU = mybir.AluOpType
    AX = mybir.AxisListType
    DR = mybir.MatmulPerfMode.DoubleRow

    nc = bacc.Bacc(
        "TRN2", target_bir_lowering=False, debug=False, num_devices=N_CORES
    )

    # Timing NEFFs keep the big tensors in Internal DRAM scratchpad so no
    # input/output bytes cross the axon tunnel; the per-iteration
    # instruction stream (and every DMA address pattern) is identical to
    # the correctness NEFF.
    kin = "Internal" if timing else "ExternalInput"
    kout = "Internal" if timing else "ExternalOutput"
    # natural-layout fp8 copy of x: F-phase moving operand
    x_dram = nc.dram_tensor("x", [BPC, C, N], dt.float8e4, kind=kin)
    # xt is host-pre-shuffled to the exact SBUF tile layout
    # [chunk, partition, k_local, c] so each chunk loads as one fully
    # contiguous 256 KB DMA (512-byte bursts otherwise).
    xt_dram = nc.dram_tensor(
        "xt", [BPC, XT_CH, P, KT // XT_CH, C], dt.float8e4, kind=kin
    )
    beta_dram = nc.dram_tensor("beta", [1, 1], dt.float32, kind="ExternalInput")
    out_dram = nc.dram_tensor("out", [BPC, C, N], dt.float8e4, kind=kout)
    tdum_dram = (
        nc.dram_tensor("tdum", [1, 1], dt.float32, kind="ExternalOutput")
        if timing
        else None
    )

    with tile.TileContext(nc) as tc, ExitStack() as ctx:
        const_pool = ctx.enter_context(tc.tile_pool(name="const", bufs=1))
        x_pool = ctx.enter_context(tc.tile_pool(name="x", bufs=3))
        xt_pool = ctx.enter_context(tc.tile_pool(name="xt", bufs=3 * XT_CH))
        f_pool = ctx.enter_context(tc.tile_pool(name="f", bufs=8))
        a_pool = ctx.enter_context(tc.tile_pool(name="a", bufs=2))
        at_pool = ctx.enter_context(tc.tile_pool(name="at", bufs=8))
        st_pool = ctx.enter_context(tc.tile_pool(name="st", bufs=2))
        spsum = ctx.enter_context(
            tc.tile_pool(name="spsum", bufs=3, space=bass.MemorySpace.PSUM)
        )
        tpsum = ctx.enter_context(
            tc.tile_pool(name="tpsum", bufs=2, space=bass.MemorySpace.PSUM)
        )
        fpsum = ctx.enter_context(
            tc.tile_pool(name="fpsum", bufs=3, space=bass.MemorySpace.PSUM)
        )

        ident = const_pool.tile([P, P], dt.bfloat16)
        masks.make_identity(nc, ident[:])

        # Broadcast beta scalar to all 128 partitions via ones.T @ beta.
        ones = const_pool.tile([1, P], dt.float32)
        nc.gpsimd.memset(ones[:], 1.0)
        beta_sb = const_pool.tile([1, 1], dt.float32)
        nc.sync.dma_start(beta_sb[:], beta_dram[:])
        beta_ps = spsum.tile([P, 1], dt.float32, tag="s_ps")
        nc.tensor.matmul(beta_ps[:], ones[:], beta_sb[:], start=True, stop=True)
        beta128 = const_pool.tile([P, 1], dt.float32)
        nc.scalar.copy(beta128[:], beta_ps[:])

        zero_f = const_pool.tile([P, N], dt.float8e4)
        if dma_only:
            nc.gpsimd.memset(zero_f[:], 0.0)

        r2 = 2 if mm2x else 1

        def emit_dma_batch(b):
            xtt = xt_pool.tile([P, XT_CH, KT // XT_CH, C], dt.float8e4, tag="xtt")
            for ch in range(XT_CH):
                nc.sync.dma_start(xtt[:, ch], xt_dram[b, ch])
            x_src = x_dram[b].rearrange("(m p) n -> p m n", p=P)
            x8 = x_pool.tile([P, MT, N], dt.float8e4)
            for m in range(MT):
                nc.sync.dma_start(x8[:, m, :], x_src[:, m, :])
            out_dst = out_dram[b].rearrange("(m p) n -> p m n", p=P)
            for m in range(MT):
                nc.sync.dma_start(out_dst[:, m, :], zero_f[:])

        def emit_head(b):
            # ---- loads ----
            # xT in XT_CH separate dma chunks so S-phase can start on the
            # first landed chunk instead of waiting for the full 2 MiB.
            xtt = []
            for ch in range(XT_CH):
                t = xt_pool.tile([P, KT // XT_CH, C], dt.float8e4, tag="xtt")
                nc.sync.dma_start(t[:], xt_dram[b, ch])
                xtt.append(t)
            x_src = x_dram[b].rearrange("(m p) n -> p m n", p=P)
            x8 = x_pool.tile([P, MT, N], dt.float8e4)
            for m in range(MT):
                nc.sync.dma_start(x8[:, m, :], x_src[:, m, :])

            # ---- S = x @ x.T  (fp8 DoubleRow, K=256 per instruction) ----
            s_tiles = []
            for m in range(MT):
                s_ps = spsum.tile([P, 512], dt.float32, tag="s_ps")
                for r in range(r2):
                    for ch in range(XT_CH):
                        for h in range(2):
                            nc.tensor.matmul(
                                s_ps[:],
                                xtt[ch][:, 2 * h : 2 * h + 2, P * m : P * (m + 1)],
                                xtt[ch][:, 2 * h : 2 * h + 2, :],
                                start=(r == 0 and ch == 0 and h == 0),
                                stop=(
                                    r == r2 - 1 and ch == XT_CH - 1 and h == 1
                                ),
                                perf_mode=DR,
                            )
                s_tiles.append(s_ps)

            # ---- softmax: E = exp(minrow - S); rzb = beta / Z folded into
            # the f_ps -> feat copy (per-partition scale after transpose) ----
            a_sb = a_pool.tile([P, MT, 512], dt.bfloat16)
            minr = st_pool.tile([P, MT], dt.float32, tag="minr")
            zsum = st_pool.tile([P, MT], dt.float32, tag="z")
            rzb = st_pool.tile([P, MT], dt.float32, tag="rzb")
            for m in range(MT):
                nc.vector.tensor_reduce(
                    minr[:, m : m + 1], s_tiles[m][:], axis=AX.X, op=ALU.min
                )
                nc.scalar.activation(
                    a_sb[:, m, :],
                    s_tiles[m][:],
                    AF.Exp,
                    bias=minr[:, m : m + 1],
                    scale=-1.0,
                    accum_out=zsum[:, m : m + 1],
                )
                nc.vector.reciprocal(rzb[:, m : m + 1], zsum[:, m : m + 1])
                nc.vector.tensor_mul(
                    rzb[:, m : m + 1], rzb[:, m : m + 1], beta128[:]
                )

            # ---- transpose E's blocks; at tiles are per-m so F(m) is gated
            # only on its own four transposed blocks. Copies on DVE keep the
            # ACT queue clear for the next batch's exps. ----
            ats = []
            for m in range(MT):
                at_m = at_pool.tile([P, KD, P], dt.float8e4, tag="at")
                for j in range(KD):
                    t_ps = tpsum.tile([P, P], dt.bfloat16, tag="t_ps")
                    nc.tensor.transpose(
                        t_ps[:], a_sb[:, m, P * j : P * (j + 1)], ident[:]
                    )
                    nc.vector.tensor_copy(at_m[:, j, :], t_ps[:])
                ats.append(at_m)
            return b, x8, ats, rzb

        def emit_tail(ctx):
            b, x8, ats, rzb = ctx
            out_dst = out_dram[b].rearrange("(m p) n -> p m n", p=P)
            for m in range(MT):
                at_m = ats[m]
                feat = f_pool.tile([P, N], dt.float8e4, tag="feat")
                for q in range(NQ):
                    f_ps = fpsum.tile([P, 512], dt.float32, tag="f_ps")
                    for r in range(r2):
                        for u in range(2):
                            nc.tensor.matmul(
                                f_ps[:],
                                at_m[:, 2 * u : 2 * u + 2, :],
                                x8[:, 2 * u : 2 * u + 2, 512 * q : 512 * (q + 1)],
                                start=(r == 0 and u == 0),
                                stop=(r == r2 - 1 and u == 1),
                                perf_mode=DR,
                            )
                    if q % 4 == 0:
                        nc.scalar.activation(
                            feat[:, 512 * q : 512 * (q + 1)],
                            f_ps[:],
                            AF.Copy,
                            scale=rzb[:, m : m + 1],
                        )
                    else:
                        nc.vector.tensor_scalar_mul(
                            feat[:, 512 * q : 512 * (q + 1)],
                            f_ps[:],
                            rzb[:, m : m + 1],
                        )
                if not no_store:
                    nc.sync.dma_start(out_dst[:, m, :], feat[:])

        def emit_body():
            # 1-deep software pipeline over the flattened batch sequence:
            # head(i+1) is emitted before tail(i) so the next batch's S
            # matmuls and softmax chain are scheduled ahead of the current
            # batch's F phase on every engine queue.
            if dma_only:
                for _ in range(reps):
                    for b in range(BPC):
                        emit_dma_batch(b)
                return
            for _ in range(reps):
                for b in range(BPC):
                    emit_tail(emit_head(b))

        if loop_iters > 1:
            with tc.For_i(0, loop_iters, 1):
                emit_body()
        else:
            emit_body()

        if timing:
            nc.sync.dma_start(tdum_dram[:], beta_sb[:])

    nc.compile()
    return nc


def _get_nc(reps=1, loop_iters=1, dma_only=False, timing=False, mm2x=False,
            no_store=False):
    key = ("nc", reps, loop_iters, dma_only, timing, mm2x, no_store)
    if key not in _CACHE:
        _CACHE[key] = _build_bass(
            reps, loop_iters, dma_only, timing, mm2x, no_store
        )
    return _CACHE[key]


def _make_in_maps(x, beta):
    x = np.ascontiguousarray(x, dtype=np.float32)
    x8 = x.astype(ml_dtypes.float8_e4m3)
    xt8 = np.ascontiguousarray(
        x.transpose(0, 2, 1), dtype=np.float32
    ).astype(ml_dtypes.float8_e4m3)
    # reorder to the kernel's SBUF tile layout: [b, ch, p, k_local, c]
    # where n = 128 * (4*ch + k_local) + p
    xt8 = np.ascontiguousarray(
        xt8.reshape(B, XT_CH, KT // XT_CH, P, C).transpose(0, 1, 3, 2, 4)
    )
    beta_arr = np.asarray(beta, dtype=np.float32).reshape(1, 1)
    in_maps = []
    for i in range(N_CORES):
        sl = slice(BPC * i, BPC * (i + 1))
        in_maps.append(
            {
                "x": np.ascontiguousarray(x8[sl]),
                "xt": np.ascontiguousarray(xt8[sl]),
                "beta": beta_arr,
            }
        )
    return in_maps


def _run(x, beta, trace=False, **kwargs):
    from concourse.bass_utils import run_bass_kernel_spmd

    x = np.ascontiguousarray(np.asarray(x), dtype=np.float32)
    nc = _get_nc()
    in_maps = _make_in_maps(x, beta)
    res = run_bass_kernel_spmd(
        nc, in_maps, core_ids=list(range(N_CORES)), trace=trace, **kwargs
    )
    feat = np.concatenate(
        [np.asarray(r["out"]).astype(np.float32) for r in res.results], axis=0
    )
    return x + feat, res


def kernel(x, beta):
    out, _ = _run(np.asarray(x), np.asarray(beta))
    return out


# revision 27
# speedup vs baseline: 1.1085x; 1.1085x over previous
"""Channel-attention kernel for Trainium2 (8 NeuronCores, SPMD data-parallel).

out[b] = beta * softmax(rowmax(S) - S, axis=-1) @ x[b] + x[b],  S = x[b] @ x[b].T

Sharding: batch dim B=16 split as 2 batches per core across 8 cores.

The device computes feat = beta * softmax(...) @ x and returns it in bf16;
the final `+ x` runs on host in fp32 (exact), so when beta == 0 the kernel
returns x bit-exactly.  Device inputs are a host-pretransposed fp8 copy xT
(the S matmul contracts over the feature dim n, which must live on SBUF
partitions for the PE) and a natural-layout fp8 copy of x (the F matmul's
moving operand).

Both matmul phases run fp8 with perf_mode=DoubleRow: operands are viewed
as [128, 2, F] APs pairing adjacent k-tiles, so one instruction contracts
K=256 (half the matmul count of bf16).

Math note: softmax(max_row(S) - S) row-wise equals exp(minrow - S) / Z with
Z = sum_d exp(minrow - S).  beta/Z is folded into A before the second
matmul.
"""

from contextlib import ExitStack

import numpy as np
import ml_dtypes

N_CORES = 8
B, C, N = 16, 512, 4096
BPC = B // N_CORES  # batches per core
P = 128
MT = C // P  # 4 row-blocks of channels
KT = N // P  # 32 partition-tiles of xT
XT_CH = 8  # xT dma/dep chunks (4 k-tiles each)
NQ = N // 512  # 8 n-chunks for the second matmul
KD = C // P  # 4 d-chunks for the second matmul

_CACHE = {}


def _build_bass(reps=1, loop_iters=1, dma_only=False, timing=False, mm2x=False,
                no_store=False):
    import concourse.bass as bass
    import concourse.bacc as bacc
    import concourse.mybir as mybir
    from concourse import tile, masks

    dt = mybir.dt
    AF = mybir.ActivationFunctionType
    ALU = mybir.AluOpType
    AX = mybir.AxisListType
    DR = mybir.MatmulPerfMode.DoubleRow

    nc = bacc.Bacc(
        "TRN2", target_bir_lowering=False, debug=False, num_devices=N_CORES
    )

    # Timing NEFFs keep the big tensors in Internal DRAM scratchpad so no
    # input/output bytes cross the axon tunnel; the per-iteration
    # instruction stream (and every DMA address pattern) is identical to
    # the correctness NEFF.
    kin = "Internal" if timing else "ExternalInput"
    kout = "Internal" if timing else "ExternalOutput"
    # natural-layout fp8 copy of x: F-phase moving operand
    x_dram = nc.dram_tensor("x", [BPC, C, N], dt.float8e4, kind=kin)
    # xt is host-pre-shuffled to the exact SBUF tile layout
    # [chunk, partition, k_local, c] so each chunk loads as one fully
    # contiguous 256 KB DMA (512-byte bursts otherwise).
    xt_dram = nc.dram_tensor(
        "xt", [BPC, XT_CH, P, KT // XT_CH, C], dt.float8e4, kind=kin
    )
    beta_dram = nc.dram_tensor("beta", [1, 1], dt.float32, kind="ExternalInput")
    out_dram = nc.dram_tensor("out", [BPC, C, N], dt.float8e4, kind=kout)
    tdum_dram = (
        nc.dram_tensor("tdum", [1, 1], dt.float32, kind="ExternalOutput")
        if timing
        else None
    )

    with tile.TileContext(nc) as tc, ExitStack() as ctx:
        const_pool = ctx.enter_context(tc.tile_pool(name="const", bufs=1))
        x_pool = ctx.enter_context(tc.tile_pool(name="x", bufs=3))
        xt_pool = ctx.enter_context(tc.tile_pool(name="xt", bufs=3 * XT_CH))
        f_pool = ctx.enter_context(tc.tile_pool(name="f", bufs=8))
        a_pool = ctx.enter_context(tc.tile_pool(name="a", bufs=2))
        at_pool = ctx.enter_context(tc.tile_pool(name="at", bufs=8))
        st_pool = ctx.enter_context(tc.tile_pool(name="st", bufs=2))
        spsum = ctx.enter_context(
            tc.tile_pool(name="spsum", bufs=3, space=bass.MemorySpace.PSUM)
        )
        tpsum = ctx.enter_context(
            tc.tile_pool(name="tpsum", bufs=2, space=bass.MemorySpace.PSUM)
        )
        fpsum = ctx.enter_context(
            tc.tile_pool(name="fpsum", bufs=3, space=bass.MemorySpace.PSUM)
        )

        ident = const_pool.tile([P, P], dt.bfloat16)
        masks.make_identity(nc, ident[:])

        # Broadcast beta scalar to all 128 partitions via ones.T @ beta.
        ones = const_pool.tile([1, P], dt.float32)
        nc.gpsimd.memset(ones[:], 1.0)
        beta_sb = const_pool.tile([1, 1], dt.float32)
        nc.sync.dma_start(beta_sb[:], beta_dram[:])
        beta_ps = spsum.tile([P, 1], dt.float32, tag="s_ps")
        nc.tensor.matmul(beta_ps[:], ones[:], beta_sb[:], start=True, stop=True)
        beta128 = const_pool.tile([P, 1], dt.float32)
        nc.scalar.copy(beta128[:], beta_ps[:])

        zero_f = const_pool.tile([P, N], dt.float8e4)
        if dma_only:
            nc.gpsimd.memset(zero_f[:], 0.0)

        r2 = 2 if mm2x else 1

        def emit_dma_batch(b):
            xtt = xt_pool.tile([P, XT_CH, KT // XT_CH, C], dt.float8e4, tag="xtt")
            for ch in range(XT_CH):
                nc.sync.dma_start(xtt[:, ch], xt_dram[b, ch])
            x_src = x_dram[b].rearrange("(m p) n -> p m n", p=P)
            x8 = x_pool.tile([P, MT, N], dt.float8e4)
            for m in range(MT):
                nc.sync.dma_start(x8[:, m, :], x_src[:, m, :])
            out_dst = out_dram[b].rearrange("(m p) n -> p m n", p=P)
            for m in range(MT):
                nc.sync.dma_start(out_dst[:, m, :], zero_f[:])

        def emit_head(b):
            # ---- loads ----
            # xT in XT_CH separate dma chunks so S-phase can start on the
            # first landed chunk instead of waiting for the full 2 MiB.
            xtt = []
            for ch in range(XT_CH):
                t = xt_pool.tile([P, KT // XT_CH, C], dt.float8e4, tag="xtt")
                nc.sync.dma_start(t[:], xt_dram[b, ch])
                xtt.append(t)
            x_src = x_dram[b].rearrange("(m p) n -> p m n", p=P)
            x8 = x_pool.tile([P, MT, N], dt.float8e4)
            for m in range(MT):
                nc.sync.dma_start(x8[:, m, :], x_src[:, m, :])

            # ---- S = x @ x.T  (fp8 DoubleRow, K=256 per instruction) ----
            s_tiles = []
            for m in range(MT):
                s_ps = spsum.tile([P, 512], dt.float32, tag="s_ps")
                for r in range(r2):
                    for ch in range(XT_CH):
                        for h in range(2):
                            nc.tensor.matmul(
                                s_ps[:],
                                xtt[ch][:, 2 * h : 2 * h + 2, P * m : P * (m + 1)],
                                xtt[ch][:, 2 * h : 2 * h + 2, :],
                                start=(r == 0 and ch == 0 and h == 0),
                                stop=(
                                    r == r2 - 1 and ch == XT_CH - 1 and h == 1
                                ),
                                perf_mode=DR,
                            )
                s_tiles.append(s_ps)

            # ---- softmax: E = exp(minrow - S); rzb = beta / Z folded into
            # the f_ps -> feat copy (per-partition scale after transpose) ----
            a_sb = a_pool.tile([P, MT, 512], dt.bfloat16)
            minr = st_pool.tile([P, MT], dt.float32, tag="minr")
            zsum = st_pool.tile([P, MT], dt.float32, tag="z")
            rzb = st_pool.tile([P, MT], dt.float32, tag="rzb")
            for m in range(MT):
                nc.vector.tensor_reduce(
                    minr[:, m : m + 1], s_tiles[m][:], axis=AX.X, op=ALU.min
                )
                nc.scalar.activation(
                    a_sb[:, m, :],
                    s_tiles[m][:],
                    AF.Exp,
                    bias=minr[:, m : m + 1],
                    scale=-1.0,
                    accum_out=zsum[:, m : m + 1],
                )
                nc.vector.reciprocal(rzb[:, m : m + 1], zsum[:, m : m + 1])
                nc.vector.tensor_mul(
                    rzb[:, m : m + 1], rzb[:, m : m + 1], beta128[:]
                )

            # ---- transpose E's blocks; at tiles are per-m so F(m) is gated
            # only on its own four transposed blocks. Copies on DVE keep the
            # ACT queue clear for the next batch's exps. ----
            ats = []
            for m in range(MT):
                at_m = at_pool.tile([P, KD, P], dt.float8e4, tag="at")
                for j in range(KD):
                    t_ps = tpsum.tile([P, P], dt.bfloat16, tag="t_ps")
                    nc.tensor.transpose(
                        t_ps[:], a_sb[:, m, P * j : P * (j + 1)], ident[:]
                    )
                    nc.vector.tensor_copy(at_m[:, j, :], t_ps[:])
                ats.append(at_m)
            return b, x8, ats, rzb

        def emit_tail(ctx):
            b, x8, ats, rzb = ctx
            out_dst = out_dram[b].rearrange("(m p) n -> p m n", p=P)
            for m in range(MT):
                at_m = ats[m]
                feat = f_pool.tile([P, N], dt.float8e4, tag="feat")
                for q in range(NQ):
                    f_ps = fpsum.tile([P, 512], dt.float32, tag="f_ps")
                    for r in range(r2):
                        for u in range(2):
                            nc.tensor.matmul(
                                f_ps[:],
                                at_m[:, 2 * u : 2 * u + 2, :],
                                x8[:, 2 * u : 2 * u + 2, 512 * q : 512 * (q + 1)],
                                start=(r == 0 and u == 0),
                                stop=(r == r2 - 1 and u == 1),
                                perf_mode=DR,
                            )
                    if q % 4 != 0:
                        nc.scalar.activation(
                            feat[:, 512 * q : 512 * (q + 1)],
                            f_ps[:],
                            AF.Copy,
                            scale=rzb[:, m : m + 1],
                        )
                    else:
                        nc.vector.tensor_scalar_mul(
                            feat[:, 512 * q : 512 * (q + 1)],
                            f_ps[:],
                            rzb[:, m : m + 1],
                        )
                if not no_store:
                    nc.sync.dma_start(out_dst[:, m, :], feat[:])

        def emit_body():
            # 1-deep software pipeline over the flattened batch sequence:
            # head(i+1) is emitted before tail(i) so the next batch's S
            # matmuls and softmax chain are scheduled ahead of the current
            # batch's F phase on every engine queue.
            if dma_only:
                for _ in range(reps):
                    for b in range(BPC):
                        emit_dma_batch(b)
                return
            for _ in range(reps):
                for b in range(BPC):
                    emit_tail(emit_head(b))

        if loop_iters > 1:
            with tc.For_i(0, loop_iters, 1):
                emit_body()
        else:
            emit_body()

        if timing:
            nc.sync.dma_start(tdum_dram[:], beta_sb[:])

    nc.compile()
    return nc


def _get_nc(reps=1, loop_iters=1, dma_only=False, timing=False, mm2x=False,
            no_store=False):
    key = ("nc", reps, loop_iters, dma_only, timing, mm2x, no_store)
    if key not in _CACHE:
        _CACHE[key] = _build_bass(
            reps, loop_iters, dma_only, timing, mm2x, no_store
        )
    return _CACHE[key]


def _make_in_maps(x, beta):
    x = np.ascontiguousarray(x, dtype=np.float32)
    x8 = x.astype(ml_dtypes.float8_e4m3)
    xt8 = np.ascontiguousarray(
        x.transpose(0, 2, 1), dtype=np.float32
    ).astype(ml_dtypes.float8_e4m3)
    # reorder to the kernel's SBUF tile layout: [b, ch, p, k_local, c]
    # where n = 128 * (4*ch + k_local) + p
    xt8 = np.ascontiguousarray(
        xt8.reshape(B, XT_CH, KT // XT_CH, P, C).transpose(0, 1, 3, 2, 4)
    )
    beta_arr = np.asarray(beta, dtype=np.float32).reshape(1, 1)
    in_maps = []
    for i in range(N_CORES):
        sl = slice(BPC * i, BPC * (i + 1))
        in_maps.append(
            {
                "x": np.ascontiguousarray(x8[sl]),
                "xt": np.ascontiguousarray(xt8[sl]),
                "beta": beta_arr,
            }
        )
    return in_maps


def _run(x, beta, trace=False, **kwargs):
    from concourse.bass_utils import run_bass_kernel_spmd

    x = np.ascontiguousarray(np.asarray(x), dtype=np.float32)
    nc = _get_nc()
    in_maps = _make_in_maps(x, beta)
    res = run_bass_kernel_spmd(
        nc, in_maps, core_ids=list(range(N_CORES)), trace=trace, **kwargs
    )
    feat = np.concatenate(
        [np.asarray(r["out"]).astype(np.float32) for r in res.results], axis=0
    )
    return x + feat, res


def kernel(x, beta):
    out, _ = _run(np.asarray(x), np.asarray(beta))
    return out
